# revision 11
# baseline (speedup 1.0000x reference)
"""BiModalAttention Trainium2 kernel (v5).

Full-input contract: kernel(mode1, mode2) -> [S, B, 2D] float32.
mode1/mode2: [S=1024, B=32, D=1024] float32.

Reference computation per batch b (m1 = mode1[:, b, :], m2 = mode2[:, b, :]):
    C1 = m1 @ m2.T                  # [S, S]
    a1 = softmax_rows(C1) @ m2 * m1
    a2 = softmax_rows(C1.T) @ m1 * m2
    out[:, b, :] = concat([a1, a2], -1)

Sharding: batch dim across 8 NeuronCores (4 batch elements per core).

Numerics (identical to the 4.3e-3-rel-err baseline scheme):
  E1[s,t] = exp(C1 - rm1[s])  (ACT, per-partition bias, fused Z1 accum)
  E2[t,s] = exp(C2 - rm2[t])  (ACT from transpose PSUM, fused Z2 accum)
  e1 = T(E1), e2 = T(E2)      (bf16 PE transposes -- exact)
  o1 = e1.T @ m2 * invZ1 * m1;  o2 = e2.T @ m1 * invZ2 * m2

v5 scheduling (driven by v4 NTFF trace: 61us HAM half-clock + 70us PE gaps):
  - [P,1024] 2-bank PSUM tiles for C1 score groups and C2 transpose
    strips: ONE 1024-wide reduce + ONE 1024-wide exp per strip instead of
    2x512 partials + min/add combines. Halves the per-strip drain chain
    that was pacing the transpose phase.
  - E1 exp emitted inside P1 right after each c1 evac: all E1 strips are
    ready ~1us after the last C1 matmul, so the e1 = T(E1) transposes
    never stall.
  - AV is split o1-first: [t-loop][o1: 16 groups][E2T+e2 copies][C1(j+1)]
    [o2: 16 groups]. The e2 ACT copies get ~40us of PE runway (o1 + C1)
    instead of 3us -- this was a 2.3us/batch stall in v4.
  - rhs chunk DMAs for batch j+1 issue at the start of o2(j), ~43us of
    runway, ring bufs=4.
  - Startup: m1t on the gpsimd DMA queue, m2t on the scalar queue --
    parallel loads halve the 22us serial input-DMA ramp.
  - PSUM: c2 tag [P,1024]x2 (4 banks), av tag [P,512]x4 (4 banks, shared
    by AV groups / E1T / E2T / keepers).
"""

import os
os.environ.setdefault("NEURON_RT_RESET_CORES", "1")
import time

import numpy as np
import ml_dtypes

import concourse.bacc as bacc
import concourse.mybir as mybir
import concourse.tile as tile
from concourse.masks import make_identity
from concourse.bass_utils import run_bass_kernel_spmd

S = 1024
D = 1024
B = 32
N_CORES = 8
BPC = B // N_CORES          # batch elements per core
P = 128                     # partitions
NK = S // P                 # contraction tiles (8)
NI = S // P                 # s tiles (8)
CW = 512                    # AV d-chunk width (bf16 matmul moving dim)
NCH = D // CW               # AV chunks (2)

f32 = mybir.dt.float32
f32r = mybir.dt.float32r
bf16 = mybir.dt.bfloat16
AX = mybir.AxisListType
ALU = mybir.AluOpType
ACTF = mybir.ActivationFunctionType


def _emit_p1(nc, sb, ps, st, j, m1t, m2t):
    # ---- Phase 1: C1 scores (fp32r) + E1 exp fused per strip ----
    m1t_sb = sb.tile([P, NK, S], f32r, tag="m1t", bufs=1, name=f"m1t_sb{j}")
    m2t_sb = sb.tile([P, NK, S], f32r, tag="m2t", bufs=1, name=f"m2t_sb{j}")
    # f32r tiles require the rounding cast only the gpsimd DMA queue does
    for (lo, hi) in ((0, NK // 2), (NK // 2, NK)):
        nc.gpsimd.dma_start(
            out=m1t_sb[:, lo:hi, :],
            in_=m1t[j].rearrange("(k p) s -> p k s", p=P)[:, lo:hi, :])
        nc.gpsimd.dma_start(
            out=m2t_sb[:, lo:hi, :],
            in_=m2t[j].rearrange("(k p) s -> p k s", p=P)[:, lo:hi, :])

    c1 = st["c1"] = []
    E1 = st["E1"] = []
    rm1 = st["rm1"] = sb.tile([P, NI], f32, tag="rm1", bufs=2, name=f"rm1_{j}")
    z1 = st["z1"] = sb.tile([P, NI], f32, tag="z1", bufs=2, name=f"z1_{j}")
    for i in range(NI):
        c1_i = sb.tile([P, S], f32, tag="c1", bufs=NI, name=f"c1_{j}_{i}")
        c1.append(c1_i)
        pc = ps.tile([P, S], f32, tag="c2", bufs=2, name=f"pc{j}_{i}")
        for n in range(2):
            for k in range(NK):
                nc.tensor.matmul(
                    pc[:, n * 512:(n + 1) * 512],
                    m1t_sb[:, k, i * P:(i + 1) * P],
                    m2t_sb[:, k, n * 512:(n + 1) * 512],
                    start=(k == 0),
                    stop=(k == NK - 1),
                )
        nc.scalar.copy(out=c1_i, in_=pc)
        nc.vector.tensor_reduce(rm1[:, i:i + 1], c1_i, axis=AX.X,
                                op=ALU.max, negate=True)
        E1_i = sb.tile([P, S], bf16, tag="E1", bufs=NI, name=f"E1_{j}_{i}")
        E1.append(E1_i)
        nc.scalar.activation(E1_i, c1_i, ACTF.Exp, bias=rm1[:, i:i + 1],
                             accum_out=z1[:, i:i + 1])


def _keeper(nc, ps, kc, nm):
    # tiny discarded bf16 matmul: keeps the PE HAM activity window busy so
    # the clock gate stays at 8/8 through softmax phases
    pk = ps.tile([P, 512], f32, tag="av", bufs=4, name=nm)
    nc.tensor.matmul(pk, kc[:, 0:P], kc, start=True, stop=True)


def _emit_p2a(nc, sb, ps, ident, identb, kc, st, j):
    """C2 transposes -> rm2 -> E2 (kept); e1 = T(E1) as PE filler."""
    c1, E1, rm1 = st["c1"], st["E1"], st["rm1"]

    invz1 = st["invz1"] = sb.tile([P, NI], f32, tag="invz1", bufs=2, name=f"invz1_{j}")
    nc.vector.reciprocal(invz1, st["z1"])

    e1 = st["e1"] = []
    rm2 = st["rm2"] = sb.tile([P, NK], f32, tag="rm2", bufs=2, name=f"rm2_{j}")
    z2 = st["z2"] = sb.tile([P, NK], f32, tag="z2", bufs=2, name=f"z2_{j}")
    E2 = st["E2"] = []
    for t in range(NK):
        pt = ps.tile([P, S], f32, tag="c2", bufs=2, name=f"pc2_{j}_{t}")
        for i in range(NI):
            nc.tensor.transpose(pt[:, i * P:(i + 1) * P],
                                c1[i][:, t * P:(t + 1) * P], ident)
        nc.vector.tensor_reduce(rm2[:, t:t + 1], pt, axis=AX.X,
                                op=ALU.max, negate=True)

        # e1_t = T(E1)[t]: no upstream deps -> PE filler while the
        # rm2/E2 chain for this t runs on DVE/ACT
        pe1 = ps.tile([P, 512], f32, tag="av", bufs=4, name=f"pe1_{j}_{t}")
        pe1b = pe1.bitcast(bf16)
        for i in range(NI):
            nc.tensor.transpose(pe1b[:, i * P:(i + 1) * P],
                                E1[i][:, t * P:(t + 1) * P], identb)
        e1_t = sb.tile([P, S], bf16, tag="e1", bufs=NK, name=f"e1_{j}_{t}")
        e1.append(e1_t)
        nc.scalar.copy(out=e1_t, in_=pe1b)

        E2_t = sb.tile([P, S], bf16, tag="E2", bufs=NK, name=f"E2_{j}_{t}")
        E2.append(E2_t)
        nc.scalar.activation(E2_t, pt, ACTF.Exp, bias=rm2[:, t:t + 1],
                             accum_out=z2[:, t:t + 1])
        if t % 2 == 0:
            _keeper(nc, ps, kc, f"kp1_{j}_{t}")


def _emit_p2b(nc, sb, ps, identb, st, j):
    """e2 = T(E2) bf16 transposes + invz2. Evacuation copies alternate
    between Scalar and Vector so the drain rate matches the PE fill rate
    (a lone ACT at ~1.1us/copy paced this phase at 2x its floor in v6)."""
    E2 = st["E2"]
    e2 = st["e2"] = []
    for i in range(NI):
        pe2 = ps.tile([P, 512], f32, tag="av", bufs=4, name=f"pe2_{j}_{i}")
        pe2b = pe2.bitcast(bf16)
        for t in range(NK):
            nc.tensor.transpose(pe2b[:, t * P:(t + 1) * P],
                                E2[t][:, i * P:(i + 1) * P], identb)
        e2_i = sb.tile([P, S], bf16, tag="e2", bufs=NI, name=f"e2_{j}_{i}")
        e2.append(e2_i)
        if i % 2 == 0:
            nc.scalar.copy(out=e2_i, in_=pe2b)
        else:
            nc.vector.tensor_copy(e2_i, pe2b)
    invz2 = st["invz2"] = sb.tile([P, NI], f32, tag="invz2", bufs=2, name=f"invz2_{j}")
    nc.vector.reciprocal(invz2, st["z2"])


def _emit_rhs_dma(nc, sb, st, j, m1n, m2n):
    """AV chunk loads for batch j (r2=mode2 chunks, r1=mode1 chunks)."""
    rts = st["rts"] = []
    for c in range(NCH):
        c0 = c * CW
        r2 = sb.tile([P, NK, CW], bf16, tag="rhs", bufs=4, name=f"r2_{j}_{c}")
        r1 = sb.tile([P, NK, CW], bf16, tag="rhs", bufs=4, name=f"r1_{j}_{c}")
        nc.gpsimd.dma_start(
            out=r2, in_=m2n[j].rearrange("(k p) d -> p k d", p=P)[:, :, c0:c0 + CW])
        nc.gpsimd.dma_start(
            out=r1, in_=m1n[j].rearrange("(k p) d -> p k d", p=P)[:, :, c0:c0 + CW])
        rts.append((r1, r2))


def _emit_p3(nc, sb, ps, st, j, outp, direction):
    """One AV direction: i-outer, chunk-inner. Both 512-wide chunks of an
    (i, direction) output land in one [P, D] staging tile so each HBM
    store writes 4KB-contiguous rows, and stores alternate between the
    sync and scalar DMA queues (a single queue sustains only ~256KB/2us,
    which backpressured stt -> PSUM -> PE in v5)."""
    if direction == 0:
        es, invz, dbase = st["e1"], st["invz1"], 0
    else:
        es, invz, dbase = st["e2"], st["invz2"], D
    for i in range(NI):
        a_sb = sb.tile([P, D], f32, tag="ao", bufs=3,
                       name=f"a{j}_{i}_{direction}")
        for c in range(NCH):
            c0 = c * CW
            r1, r2 = st["rts"][c]
            rhs, gate = (r2, r1) if direction == 0 else (r1, r2)
            pav = ps.tile([P, CW], f32, tag="av", bufs=4,
                          name=f"pav{j}_{c}_{i}_{direction}")
            for k in range(NK):
                nc.tensor.matmul(
                    pav,
                    es[k][:, i * P:(i + 1) * P],
                    rhs[:, k, :],
                    start=(k == 0),
                    stop=(k == NK - 1),
                )
            nc.vector.scalar_tensor_tensor(
                a_sb[:, c0:c0 + CW], pav, invz[:, i:i + 1],
                gate[:, i, :],
                op0=ALU.mult, op1=ALU.mult)
        q = nc.sync if i % 2 == 0 else nc.scalar
        q.dma_start(
            out=outp[j, i * P:(i + 1) * P, dbase:dbase + D],
            in_=a_sb)


def _build():
    nc = bacc.Bacc("TRN2", target_bir_lowering=False, debug=False,
                   num_devices=N_CORES)
    m1n = nc.dram_tensor("m1n", [BPC, S, D], bf16, kind="ExternalInput").ap()
    m2n = nc.dram_tensor("m2n", [BPC, S, D], bf16, kind="ExternalInput").ap()
    m1t = nc.dram_tensor("m1t", [BPC, D, S], f32, kind="ExternalInput").ap()
    m2t = nc.dram_tensor("m2t", [BPC, D, S], f32, kind="ExternalInput").ap()
    outp = nc.dram_tensor("out", [BPC, S, 2 * D], f32, kind="ExternalOutput").ap()

    with tile.TileContext(nc) as tc:
        with tc.tile_pool(name="consts", bufs=1) as consts, \
             tc.tile_pool(name="sb", bufs=1) as sb, \
             tc.tile_pool(name="ps", bufs=1, space="PSUM") as ps:
            ident = consts.tile([P, P], f32)
            make_identity(nc, ident)
            identb = consts.tile([P, P], bf16)
            make_identity(nc, identb)
            kc = consts.tile([P, 512], bf16)
            nc.vector.memset(kc, 1.0)
            # Pipeline per batch j:
            #   [P2a(j): C2T+E1T t-loop][o1(j): 16 AV groups]
            #   [P2b(j): E2T + e2 copies][P1(j+1): C1 scores]
            #   [rhs DMA (j+1)][o2(j): 16 AV groups]
            # C1(j+1) and o1(j) give the softmax/evac chains of batch j
            # PE runway; rhs DMAs lead their consumers by ~2 phases.
            sts = [dict() for _ in range(BPC)]
            _emit_p1(nc, sb, ps, sts[0], 0, m1t, m2t)
            _emit_rhs_dma(nc, sb, sts[0], 0, m1n, m2n)
            for j in range(BPC):
                _emit_p2a(nc, sb, ps, ident, identb, kc, sts[j], j)
                _emit_p3(nc, sb, ps, sts[j], j, outp, 0)
                _emit_p2b(nc, sb, ps, identb, sts[j], j)
                if j + 1 < BPC:
                    _emit_p1(nc, sb, ps, sts[j + 1], j + 1, m1t, m2t)
                    _emit_rhs_dma(nc, sb, sts[j + 1], j + 1, m1n, m2n)
                _emit_p3(nc, sb, ps, sts[j], j, outp, 1)
    nc.compile()
    return nc


_NC_CACHE = None


def _get_nc():
    global _NC_CACHE
    if _NC_CACHE is None:
        _NC_CACHE = _build()
    return _NC_CACHE


def kernel(mode1: np.ndarray, mode2: np.ndarray, _trace: bool = False,
           _result_box: dict | None = None) -> np.ndarray:
    mode1 = np.asarray(mode1, dtype=np.float32)
    mode2 = np.asarray(mode2, dtype=np.float32)

    m1n_all = np.ascontiguousarray(
        mode1.transpose(1, 0, 2)).astype(ml_dtypes.bfloat16)      # [B, S, D]
    m2n_all = np.ascontiguousarray(
        mode2.transpose(1, 0, 2)).astype(ml_dtypes.bfloat16)
    m1t_all = np.ascontiguousarray(mode1.transpose(1, 2, 0))      # [B, D, S]
    m2t_all = np.ascontiguousarray(mode2.transpose(1, 2, 0))

    nc = _get_nc()
    in_maps = []
    for c in range(N_CORES):
        lo, hi = c * BPC, (c + 1) * BPC
        in_maps.append({
            "m1n": m1n_all[lo:hi],
            "m2n": m2n_all[lo:hi],
            "m1t": m1t_all[lo:hi],
            "m2t": m2t_all[lo:hi],
        })

    r = None
    last_err = None
    for attempt in range(3):
        try:
            r = run_bass_kernel_spmd(nc, in_maps, list(range(N_CORES)),
                                     trace=_trace)
            break
        except Exception as e:  # transient NRT exec-unit errors recover on retry
            last_err = e
            time.sleep(2.0)
    if r is None:
        raise last_err
    if _result_box is not None:
        _result_box["result"] = r

    out = np.empty((S, B, 2 * D), dtype=np.float32)
    for c in range(N_CORES):
        res = r.results[c]["out"]  # [BPC, S, 2D]
        out[:, c * BPC:(c + 1) * BPC, :] = res.transpose(1, 0, 2)
    return out
